# revision 2
# baseline (speedup 1.0000x reference)
"""Trainium2 Bass kernel for the ClusterLoss problem.

Loss = mean-entropy(softmax over K of [T, M, K] logits)            (L1)
       - mean-entropy(softmax over K of batch-mean logits [M, K])  (L2)

T=4096, M=64, K=256 hardcoded. Data-parallel over T across 8 cores.

Per core (shard = [512*64, 256] rows, viewed as 256 tiles of [128, 256]):
  - DMA 1 MiB mega-tiles (8 row-tiles) into SBUF.
  - ACT:  e = exp(x) per tile slice, accum_out -> Z[row] column buffer.
  - DVE:  w = x*e (in-place over e), accum_out -> S[row] column buffer
          (fused tensor_tensor_reduce).
  - PE:   0/1-pattern matmul accumulates per-block sums over T into PSUM
          (for L2's batch-mean logits).
  - tail: H_sum/partition = sum_tiles ln(Z) - sum_tiles S*(1/Z),
          computed batched on the [128, 256] stat buffers.
Outputs per core: ent [128,1] (partition-sums of per-row entropies) and
bsum [64,256] (partial sum over this core's T rows). Host reduces those
tiny tensors into the final scalar.

No max-subtraction in the softmax: inputs are standard-normal, |x| < ~6,
so exp(x) is comfortably inside fp32 range; H = ln(Z) - S/Z is
analytically identical to the reference's log_softmax entropy.
"""

import numpy as np

import concourse.bacc as bacc
import concourse.bass as bass
import concourse.tile as tile
from concourse import mybir
from concourse.bass_utils import run_bass_kernel_spmd

T, M, K = 4096, 64, 256
NCORES = 8
TSH = T // NCORES            # 512 t-rows per core
ROWS = TSH * M               # 32768 (t, m) rows per core
P = 128                      # SBUF partitions per tile
NTILES = ROWS // P           # 256 tiles of [128, 256] per core
MEGA = 8                     # row-tiles per DMA (1 MiB transfers)
NMEGA = NTILES // MEGA       # 32
PAIR = 2                     # row-tiles per PE matmul (moving free dim 512)
BSPLIT = 3                   # per mega-tile: last BSPLIT tiles use batched
                             # exp + DVE Z-reduce (ACT/DVE load balance)
BSPLIT_ALT = True            # odd mega-tiles use BSPLIT+1 (avg +0.5)

FP32 = mybir.dt.float32
FP32R = mybir.dt.float32r


def _build_nc(use_pe=True, use_act=True, use_dve=True, repeat=1):
    from contextlib import nullcontext

    nc = bacc.Bacc("TRN2", target_bir_lowering=False, debug=False)

    x_d = nc.dram_tensor("x", [NTILES, P, K], FP32R, kind="ExternalInput")
    w_d = nc.dram_tensor("wpat", [P, M], FP32R, kind="ExternalInput")
    ent_d = nc.dram_tensor("ent", [P, 1], FP32, kind="ExternalOutput")
    bsum_d = nc.dram_tensor("bsum", [M, K], FP32, kind="ExternalOutput")

    x = x_d.ap()

    with tile.TileContext(nc) as tc:
        with (
            tc.tile_pool(name="xin", bufs=4) as xpool,
            tc.tile_pool(name="exp", bufs=4) as epool,
            tc.tile_pool(name="stats", bufs=1) as stats,
            tc.tile_pool(name="small", bufs=1) as small,
            tc.tile_pool(name="psum", bufs=1, space="PSUM") as psum,
            tc.For_i(0, repeat, 1) if repeat > 1 else nullcontext(),
        ):
            wp = small.tile([P, M], FP32R)
            nc.sync.dma_start(out=wp, in_=w_d.ap())

            # Stats columns split by mode so each buffer has one writer
            # engine (ACT writes *_a accums; DVE writes everything else).
            nb_per_mega = BSPLIT if (use_act and use_dve) else (0 if use_act else MEGA)
            n_b = NMEGA * nb_per_mega              # tiles in batched mode
            if nb_per_mega == BSPLIT and BSPLIT_ALT:
                n_b += NMEGA // 2
            n_a = NTILES - n_b
            zbuf_a = stats.tile([P, max(n_a, 1)], FP32)
            sbuf_a = stats.tile([P, max(n_a, 1)], FP32)
            zbuf_b = stats.tile([P, max(n_b, 1)], FP32)
            sbuf_b = stats.tile([P, max(n_b, 1)], FP32)
            bs_ps = psum.tile([M, PAIR, K], FP32)  # block sums (2 halves)

            ia = ib = 0
            for mg in range(NMEGA):
                xtr = xpool.tile([P, MEGA, K], FP32R)
                nc.sync.dma_start(
                    out=xtr,
                    in_=x[mg * MEGA:(mg + 1) * MEGA].rearrange("c p k -> p c k"),
                )
                xt = xtr.bitcast(FP32)
                et = epool.tile([P, MEGA, K], FP32)
                nb_here = nb_per_mega
                if nb_per_mega == BSPLIT and BSPLIT_ALT and (mg % 2):
                    nb_here = BSPLIT + 1
                na_here = MEGA - nb_here
                # mode A tiles (slices 0..na_here): fused exp+Z on ACT
                for j in range(na_here):
                    if use_act:
                        nc.scalar.activation(
                            out=et[:, j, :],
                            in_=xt[:, j, :],
                            func=mybir.ActivationFunctionType.Exp,
                            accum_out=zbuf_a[:, ia:ia + 1],
                        )
                    if use_dve:
                        nc.vector.affine_mul_reduce(
                            out=et[:, j, :],
                            accum_out=sbuf_a[:, ia:ia + 1],
                            in0=xt[:, j, :],
                            in1=et[:, j, :],
                            scale=1.0,
                            bias=0.0,
                        )
                    ia += 1
                # mode B tiles (remaining slices): one batched exp, Z on DVE
                if nb_here:
                    if use_act:
                        nc.scalar.activation(
                            out=et[:, na_here:, :], in_=xt[:, na_here:, :],
                            func=mybir.ActivationFunctionType.Exp,
                        )
                    if use_dve:
                        # all nb_here per-row Z sums in one 3D reduce
                        nc.vector.tensor_reduce(
                            out=zbuf_b[:, ib:ib + nb_here],
                            in_=et[:, na_here:, :],
                            axis=mybir.AxisListType.X,
                            op=mybir.AluOpType.add,
                        )
                    for j in range(na_here, MEGA):
                        if use_dve:
                            nc.vector.affine_mul_reduce(
                                out=et[:, j, :],
                                accum_out=sbuf_b[:, ib:ib + 1],
                                in0=xt[:, j, :],
                                in1=et[:, j, :],
                                scale=1.0,
                                bias=0.0,
                            )
                        ib += 1
                if use_pe:
                    for j in range(MEGA // PAIR):
                        g = mg * (MEGA // PAIR) + j
                        nc.tensor.matmul(
                            bs_ps,
                            wp,
                            xtr[:, j * PAIR:(j + 1) * PAIR, :],
                            start=(g == 0),
                            stop=(g == NMEGA * (MEGA // PAIR) - 1),
                        )

            # ---- tail: batched entropy math over the stat buffers ----
            parts = []
            for idx, (zb, sb, n) in enumerate(
                ((zbuf_a, sbuf_a, n_a), (zbuf_b, sbuf_b, n_b))
            ):
                if n == 0:
                    continue
                if not (use_act and use_dve):
                    nc.vector.memset(zb, 1.0)
                    nc.vector.memset(sb, 0.0)
                logz = stats.tile([P, n], FP32, tag=f"logz{idx}")
                lsum = small.tile([P, 1], FP32, tag=f"lsum{idx}")
                nc.scalar.activation(
                    out=logz, in_=zb,
                    func=mybir.ActivationFunctionType.Ln,
                    accum_out=lsum,
                )
                rz = stats.tile([P, n], FP32, tag=f"rz{idx}")
                nc.vector.reciprocal(out=rz, in_=zb)
                szsum = small.tile([P, 1], FP32, tag=f"szsum{idx}")
                nc.vector.affine_mul_reduce(
                    out=rz, accum_out=szsum,
                    in0=sb, in1=rz,
                    scale=1.0, bias=0.0,
                )
                part = small.tile([P, 1], FP32, tag=f"part{idx}")
                nc.vector.tensor_sub(part, lsum, szsum)
                parts.append(part)
            ent_sb = small.tile([P, 1], FP32)
            if len(parts) == 2:
                nc.vector.tensor_add(ent_sb, parts[0], parts[1])
            else:
                nc.vector.tensor_copy(out=ent_sb, in_=parts[0])
            nc.sync.dma_start(out=ent_d.ap(), in_=ent_sb)

            bsum_sb = small.tile([M, K], FP32)
            if use_pe:
                nc.scalar.copy(bsum_sb, bs_ps[:, 0, :])
                nc.vector.tensor_add(bsum_sb, bsum_sb, bs_ps[:, 1, :])
            else:
                nc.vector.memset(bsum_sb, 0.0)
            nc.sync.dma_start(out=bsum_d.ap(), in_=bsum_sb)

    nc.compile()
    return nc


_NC_CACHE = []


def _get_nc():
    if not _NC_CACHE:
        _NC_CACHE.append(_build_nc())
    return _NC_CACHE[0]


def _wpat():
    wp = np.zeros((P, M), np.float32)
    wp[np.arange(P), np.arange(P) % M] = 1.0
    return wp


def _input_map(shard):
    """Per-core input dict for a [NTILES, P, K] fp32 shard."""
    return {"x": np.ascontiguousarray(shard), "wpat": _wpat()}


def kernel(block_feats, **kw):
    assert int(kw.get("M", M)) == M
    xf = np.ascontiguousarray(np.asarray(block_feats, dtype=np.float32))
    assert xf.shape == (T, M * K)
    shards = xf.reshape(NCORES, NTILES, P, K)

    nc = _get_nc()
    wp = _wpat()
    in_maps = [{"x": shards[i], "wpat": wp} for i in range(NCORES)]
    res = run_bass_kernel_spmd(nc, in_maps, core_ids=list(range(NCORES))).results

    ent_total = sum(float(r["ent"].sum(dtype=np.float64)) for r in res)
    L1 = ent_total / (T * M)

    bs = np.zeros((M, K), np.float64)
    for r in res:
        bs += r["bsum"]
    bm = bs / T
    z = bm - bm.max(axis=-1, keepdims=True)
    e = np.exp(z)
    Z = e.sum(axis=-1, keepdims=True)
    logp = z - np.log(Z)
    H = -(np.exp(logp) * logp).sum(axis=-1)
    L2 = -H.mean()

    return np.asarray(L1 + L2, dtype=np.float32)



# revision 31
# speedup vs baseline: 2.3321x; 2.3321x over previous
"""Trainium2 Bass kernel for the ClusterLoss problem.

Loss = mean-entropy(softmax over K of [T, M, K] logits)            (L1)
       - mean-entropy(softmax over K of batch-mean logits [M, K])  (L2)

T=4096, M=64, K=256 hardcoded. Data-parallel over T across 8 cores.

Per core (shard = [512*64, 256] rows, viewed as 256 tiles of [128, 256]):
  - DMA 2 MiB mega-tiles (16 row-tiles) into SBUF.
  - ACT:  e = exp(x), one batched instruction per mega-tile.
  - DVE:  one custom fused op per mega-tile streams (x, e) once and
          writes BOTH running sums, multiplexed by element position:
            out[k] = (k at a block's last slot) ? cumsum(e) : cumsum(x*e)
          Per-block Z = sum(e) and S = sum(x*e) are recovered in the tail
          from first-differences of the cumulative values extracted at
          block boundaries (plus the one missing x*e term per block,
          recomputed from the extracted last-element x).
  - PE:   0/1-pattern matmul accumulates per-block sums over T into PSUM
          (for L2's batch-mean logits).
  - tail: H partition-sums = sum ln(Z) - sum S*(1/Z) over the [128, 256]
          per-(row, block) stat buffers.
Outputs per core: ent [128,1] (partition-sums of per-row entropies) and
bsum [64,256] (partial sum over this core's T rows). Host reduces those
tiny tensors into the final scalar.

No max-subtraction in the softmax: inputs are standard-normal, |x| < ~6,
so exp(x) is comfortably inside fp32 range; H = ln(Z) - S/Z is
analytically identical to the reference's log_softmax entropy.
"""

import numpy as np

import concourse.bacc as bacc
import concourse.bass as bass
import concourse.tile as tile
from concourse import mybir
from concourse.bass_utils import run_bass_kernel_spmd

T, M, K = 4096, 64, 256
NCORES = 8
TSH = T // NCORES            # 512 t-rows per core
ROWS = TSH * M               # 32768 (t, m) rows per core
P = 128                      # SBUF partitions per tile
NTILES = ROWS // P           # 256 tiles of [128, 256] per core
MEGA = 8                     # row-tiles per DMA (1 MiB transfers)
NMEGA = NTILES // MEGA       # 32
PAIR = 2                     # row-tiles per PE matmul (moving free dim 512)

FP32 = mybir.dt.float32
FP32R = mybir.dt.float32r


# --- custom DVE op: fused segmented-cumsum of (x*e, e) ---------------------- #
# out[p, s, k] = Idx >= 255+256*s ? cumsum(e)[p,s,k] : cumsum(x*e)[p,s,k]
# where both cumsums run over the whole [S*N] stream of one instruction
# (seeded per instruction, NOT per block; block values come from diffs).

def _register_fused_op():
    from concourse import dve_ops as _ops
    from concourse.dve_spec import (
        Spec, Src0, Src1, C0, C1, AluOp, scan, PageIdx, Idx, select, lower,
        _has_src1,
    )
    from concourse.dve_uop import DveOpSpec

    name = "SEGSCAN_SZ_ANT"
    for op in _ops.OPS:
        if op.name == name:
            return op

    pg = PageIdx(C0, C1)  # c0 + s*c1 ; call with s0=N-1, s1=N
    body = select(
        Idx >= pg,
        scan(AluOp.ADD, Src1),
        scan(AluOp.ADD, Src0 * Src1),
    )

    def _ref(in0, in1, c0, c1, c2):
        p = in0.shape[0]
        x = np.asarray(in0, np.float32).reshape(p, -1).astype(np.float64)
        e = np.asarray(in1, np.float32).reshape(p, -1).astype(np.float64)
        n = x.shape[1]
        start = float(np.asarray(c0).flat[0])
        step = float(np.asarray(c1).flat[0])
        idx = np.arange(n, dtype=np.float64)
        pgv = start + np.floor(idx / step) * step
        zc = np.cumsum(e, axis=1)
        sc = np.cumsum(x * e, axis=1)
        out = np.where(idx >= pgv, zc, sc).astype(np.float32)
        return out.reshape(in0.shape)

    spec = Spec(body=body, reference=_ref)
    row = _ops._CUSTOM_DVE_ROW_BASE + len(_ops.OPS)
    shas = {}
    for ver in ("v3", "v4"):
        t = DveOpSpec(
            name=name, opcode=row, uops=lower(spec, ver=ver),
            rd1_en=_has_src1(spec),
        )
        shas[ver] = t.sha(ver)
    op = _ops.DveOp(name, spec, subdim=True, uops_sha=shas)
    _ops.OPS.append(op)
    _ops.CUSTOM_DVE_SPECS[name] = spec
    _ops._SUB_OPCODE_FOR_NAME[name] = row
    return op


FUSED_SZ = _register_fused_op()


def _build_nc(use_pe=True, use_act=True, use_dve=True, repeat=1, mega=MEGA,
              xbufs=4, ebufs=4, do_op=None, do_extract=None, ext_mode=3):
    """ext_mode: bit0 = stat2 copy, bit1 = xlb copy, bit2 = copies on ACT,
    bit3 = xlb copy issued before the fused op, bit4 = dense fake copies."""
    from contextlib import nullcontext

    if do_op is None:
        do_op = use_dve
    if do_extract is None:
        do_extract = use_dve
    if not do_extract:
        ext_mode = 0
    nmega = NTILES // mega
    nc = bacc.Bacc("TRN2", target_bir_lowering=False, debug=False)

    x_d = nc.dram_tensor("x", [NTILES, P, K], FP32R, kind="ExternalInput")
    w_d = nc.dram_tensor("wpat", [P, M], FP32R, kind="ExternalInput")
    i_d = nc.dram_tensor("ident", [P, P], FP32, kind="ExternalInput")
    ent_d = nc.dram_tensor("ent", [P, 1], FP32, kind="ExternalOutput")
    bsum_d = nc.dram_tensor("bsum", [M, K], FP32, kind="ExternalOutput")

    x = x_d.ap()

    with tile.TileContext(nc) as tc:
        with (
            tc.tile_pool(name="xin", bufs=xbufs) as xpool,
            tc.tile_pool(name="exp", bufs=ebufs) as epool,
            tc.tile_pool(name="stats", bufs=1) as stats,
            tc.tile_pool(name="small", bufs=1) as small,
            tc.tile_pool(name="psum", bufs=1, space="PSUM") as psum,
            tc.For_i(0, repeat, 1) if repeat > 1 else nullcontext(),
        ):
            wp = small.tile([P, M], FP32R)
            nc.sync.dma_start(out=wp, in_=w_d.ap())
            if ext_mode & 64:
                idt = small.tile([P, P], FP32)
                nc.sync.dma_start(out=idt, in_=i_d.ap())

            # stat2[:, g, c, 0] = cum S through (block c of mega g, elem K-2)
            # stat2[:, g, c, 1] = cum Z through (block c of mega g, elem K-1)
            stat2 = stats.tile([P, nmega, mega, 2], FP32)
            xlb = stats.tile([P, nmega, mega], FP32)  # x[last elem] per block
            bs_ps = psum.tile([M, PAIR, K], FP32)  # block sums (2 halves)
            # PE-extracted stats, inner pairs (elem K-2, K-1):
            # eps2 = fused-op out -> (cumS@K-2, cumZ@K-1); eps3 = x -> (_, x@K-1)
            eps2 = psum.tile([P, nmega, mega, 2], FP32)
            eps3 = psum.tile([P, nmega, mega, 2], FP32)

            if ext_mode & 4:
                _copy = lambda out, in_: nc.scalar.copy(out, in_)
            elif ext_mode & 32:
                _copy = lambda out, in_: nc.gpsimd.tensor_copy(out=out, in_=in_)
            else:
                _copy = lambda out, in_: nc.vector.tensor_copy(out=out, in_=in_)

            def _extract(mg, xtr, et):
                xt = xtr.bitcast(FP32)
                if ext_mode & 64:
                    # PE identity matmuls: free-dim column-pair selection
                    # into PSUM, no DVE involvement.
                    nc.tensor.matmul(
                        eps2[:, mg], idt, et[:, :, K - 2:K],
                        start=True, stop=True,
                    )
                    nc.tensor.matmul(
                        eps3[:, mg], idt, xt[:, :, K - 2:K],
                        start=True, stop=True,
                    )
                    return
                if ext_mode & 1:
                    if ext_mode & 16:
                        _copy(stat2[:, mg, :, 0], et[:, 0, :mega])
                    else:
                        _copy(stat2[:, mg, :, :], et[:, :, K - 2:K])
                if ext_mode & 2:
                    _copy(
                        xlb[:, mg, :],
                        xt[:, 0, :mega] if ext_mode & 16 else xt[:, :, K - 1],
                    )

            pending = None
            for mg in range(nmega):
                xtr = xpool.tile([P, mega, K], FP32R, tag="xtr")
                nc.sync.dma_start(
                    out=xtr,
                    in_=x[mg * mega:(mg + 1) * mega].rearrange("c p k -> p c k"),
                )
                xt = xtr.bitcast(FP32)
                et = epool.tile([P, mega, K], FP32, tag="et")
                if use_act:
                    nc.scalar.activation(
                        out=et, in_=xt,
                        func=mybir.ActivationFunctionType.Exp,
                    )
                if do_op:
                    # fused cumulative (x*e, e) with positional multiplex;
                    # writes over et in place.
                    nc.vector._custom_dve(
                        FUSED_SZ, out=et, in0=xt, in1=et,
                        s0=float(K - 1), s1=float(K),
                    )
                if pending is not None:
                    _extract(*pending)
                pending = (mg, xtr, et)
                if use_pe:
                    for j in range(mega // PAIR):
                        g = mg * (mega // PAIR) + j
                        nc.tensor.matmul(
                            bs_ps,
                            wp,
                            xtr[:, j * PAIR:(j + 1) * PAIR, :],
                            start=(g == 0),
                            stop=(g == nmega * (mega // PAIR) - 1),
                        )
            if pending is not None:
                _extract(*pending)

            # ---- tail: recover per-block S/Z, then batched entropy ----
            if ext_mode & 64:
                exb = stats.tile([P, nmega, mega, 2], FP32)
                exb3 = stats.tile([P, nmega, mega, 2], FP32)
                nc.vector.tensor_copy(out=exb, in_=eps2)
                nc.vector.tensor_copy(out=exb3, in_=eps3)
                sc254 = exb[:, :, :, 0]
                zc = exb[:, :, :, 1]
                xlbv = exb3[:, :, :, 1]
            else:
                if not ext_mode & 1:
                    nc.vector.memset(stat2, 1.0)
                if not ext_mode & 2:
                    nc.vector.memset(xlb, 0.0)
                sc254, zc, xlbv = stat2[:, :, :, 0], stat2[:, :, :, 1], xlb
            elb = stats.tile([P, nmega, mega], FP32)
            if use_act:
                nc.scalar.activation(
                    out=elb, in_=xlbv, func=mybir.ActivationFunctionType.Exp
                )
            else:
                nc.vector.memset(elb, 1.0)
            # SF = true inclusive cum-S through block end = cumS254 + xl*el
            sf = stats.tile([P, nmega, mega], FP32)
            nc.vector.tensor_mul(sf, xlbv, elb)
            nc.vector.tensor_add(sf, sf, sc254)
            # per-block values: within-mega first differences
            sp = stats.tile([P, nmega, mega], FP32)
            zp = stats.tile([P, nmega, mega], FP32)
            nc.vector.tensor_copy(out=sp[:, :, 0], in_=sf[:, :, 0])
            nc.vector.tensor_copy(out=zp[:, :, 0], in_=zc[:, :, 0])
            nc.vector.tensor_sub(
                sp[:, :, 1:], sf[:, :, 1:], sf[:, :, :mega - 1]
            )
            nc.vector.tensor_sub(
                zp[:, :, 1:], zc[:, :, 1:], zc[:, :, :mega - 1]
            )
            # entropy partition-sums: ent = sum ln(zp) - sum sp/zp
            logz = stats.tile([P, nmega, mega], FP32)
            lsum = small.tile([P, 1], FP32, tag="lsum")
            nc.scalar.activation(
                out=logz, in_=zp,
                func=mybir.ActivationFunctionType.Ln,
                accum_out=lsum,
            )
            rz = stats.tile([P, nmega, mega], FP32)
            nc.vector.reciprocal_approx_accurate(
                out=rz, in_=zp, scratch=logz
            )
            szsum = small.tile([P, 1], FP32, tag="szsum")
            nc.vector.affine_mul_reduce(
                out=rz, accum_out=szsum,
                in0=sp, in1=rz,
                scale=1.0, bias=0.0,
            )
            ent_sb = small.tile([P, 1], FP32)
            nc.vector.tensor_sub(ent_sb, lsum, szsum)
            nc.sync.dma_start(out=ent_d.ap(), in_=ent_sb)

            bsum_sb = small.tile([M, K], FP32)
            if use_pe:
                nc.scalar.copy(bsum_sb, bs_ps[:, 0, :])
                nc.vector.tensor_add(bsum_sb, bsum_sb, bs_ps[:, 1, :])
            else:
                nc.vector.memset(bsum_sb, 0.0)
            nc.sync.dma_start(out=bsum_d.ap(), in_=bsum_sb)

    nc.compile()
    return nc


_NC_CACHE = []


def _get_nc():
    if not _NC_CACHE:
        _NC_CACHE.append(_build_nc())
    return _NC_CACHE[0]


def _wpat():
    wp = np.zeros((P, M), np.float32)
    wp[np.arange(P), np.arange(P) % M] = 1.0
    return wp


def _input_map(shard):
    """Per-core input dict for a [NTILES, P, K] fp32 shard."""
    return {
        "x": np.ascontiguousarray(shard),
        "wpat": _wpat(),
        "ident": np.eye(P, dtype=np.float32),
    }


def kernel(block_feats, **kw):
    assert int(kw.get("M", M)) == M
    xf = np.ascontiguousarray(np.asarray(block_feats, dtype=np.float32))
    assert xf.shape == (T, M * K)
    shards = xf.reshape(NCORES, NTILES, P, K)

    nc = _get_nc()
    in_maps = [_input_map(shards[i]) for i in range(NCORES)]
    res = run_bass_kernel_spmd(nc, in_maps, core_ids=list(range(NCORES))).results

    ent_total = sum(float(r["ent"].sum(dtype=np.float64)) for r in res)
    L1 = ent_total / (T * M)

    bs = np.zeros((M, K), np.float64)
    for r in res:
        bs += r["bsum"]
    bm = bs / T
    z = bm - bm.max(axis=-1, keepdims=True)
    e = np.exp(z)
    Z = e.sum(axis=-1, keepdims=True)
    logp = z - np.log(Z)
    H = -(np.exp(logp) * logp).sum(axis=-1)
    L2 = -H.mean()

    return np.asarray(L1 + L2, dtype=np.float32)
